# revision 1
# baseline (speedup 1.0000x reference)
"""DeepSeekMoE Trainium2 kernel (8 NeuronCores, SPMD).

Strategy:
  - Host computes top-2 routing (sharding decision only) and gathers tokens
    into per-expert groups of capacity CAP, forming a "pairs" matrix
    [D, E*CAP].  Every core receives the same pairs; the routed experts are
    tensor-parallel over d_ff: core c computes f-slice [c*512, (c+1)*512) of
    every expert's FFN for every pair, producing a partial output that the
    host reduces over cores and scatter-adds back to token positions.
  - The gate (softmax over expert logits, prob of the block's expert) is
    computed on device: bf16 logits matmul into fp32 psum, Exp on ScalarE,
    partition reductions on GpSimd/VectorE (off the PE critical path),
    software-pipelined one expert ahead of the FFN that consumes it.
  - Shared experts are sharded 1:1: core c runs shared expert c over all
    T tokens; host sums the 8 contributions.  Shared chunks are interleaved
    between routed experts to keep the PE dense.
  - All matmuls bf16 (fp32 psum).  alpha/NS is folded into w2_s on the host
    (exact power-of-two scale); (1-alpha) is folded into the gate.
"""

import contextlib

import numpy as np
import ml_dtypes

import concourse.bacc as bacc
import concourse.bass_isa as bass_isa
import concourse.tile as tile
import concourse.mybir as mybir
from concourse.bass_utils import run_bass_kernel_spmd

BF16 = ml_dtypes.bfloat16

B, S, D, F, E, NS, K = 2, 1024, 1024, 4096, 8, 8, 2
T = B * S
FS = F // NS            # shared expert hidden = 512
FL = F // 8             # per-core f-slice of routed experts = 512
KCFG = {"cap": 576, "yo_act": True, "gate_first": False}
CAP = KCFG["cap"]       # per-expert token capacity (max observed 540)
NPAIR = E * CAP
ALPHA = 0.5
N_CORES = 8

_NC = None          # compiled program cache
LAST_RESULT = None  # BassKernelResults of the most recent run (for profiling)


def _build_program(cfg=None):
    cfg = cfg or {}
    cap = cfg.get("cap", CAP)
    yo_act = cfg.get("yo_act", True)       # psum->sbuf copy engine for yr
    gate_first = cfg.get("gate_first", False)  # gate(e+1) before ffn13(e)
    chunks = [(0, 512), (512, cap - 512)] if cap > 512 else [(0, cap)]

    bf = mybir.dt.bfloat16
    f32 = mybir.dt.float32
    Act = mybir.ActivationFunctionType
    Alu = mybir.AluOpType

    nc = bacc.Bacc("TRN2", target_bir_lowering=False, debug=False,
                   num_devices=N_CORES)

    npair = E * cap
    xp = nc.dram_tensor("xp", [D, npair], bf, kind="ExternalInput").ap()
    xf = nc.dram_tensor("xf", [D, T], bf, kind="ExternalInput").ap()
    gw = nc.dram_tensor("gw", [D, E], bf, kind="ExternalInput").ap()
    sel = nc.dram_tensor("sel", [E, E], f32, kind="ExternalInput").ap()
    w1l = nc.dram_tensor("w1l", [E, D, FL], bf, kind="ExternalInput").ap()
    w3l = nc.dram_tensor("w3l", [E, D, FL], bf, kind="ExternalInput").ap()
    w2l = nc.dram_tensor("w2l", [E, FL, D], bf, kind="ExternalInput").ap()
    w1s = nc.dram_tensor("w1s", [D, FS], bf, kind="ExternalInput").ap()
    w3s = nc.dram_tensor("w3s", [D, FS], bf, kind="ExternalInput").ap()
    w2s = nc.dram_tensor("w2s", [FS, D], bf, kind="ExternalInput").ap()
    yr = nc.dram_tensor("yr", [D, npair], bf, kind="ExternalOutput").ap()
    ys = nc.dram_tensor("ys", [D, T], bf, kind="ExternalOutput").ap()

    xp_r = xp.rearrange("(a p) t -> p a t", p=128)
    xf_r = xf.rearrange("(a p) t -> p a t", p=128)
    gw_r = gw.rearrange("(a p) e -> p a e", p=128)
    yr_r = yr.rearrange("(a p) t -> p a t", p=128)
    ys_r = ys.rearrange("(a p) t -> p a t", p=128)
    w1s_r = w1s.rearrange("(a p) f -> p a f", p=128)
    w3s_r = w3s.rearrange("(a p) f -> p a f", p=128)
    w2s_r = w2s.rearrange("(a p) d -> p a d", p=128)

    with tile.TileContext(nc) as tc:
        with contextlib.ExitStack() as ctx:
            const = ctx.enter_context(tc.tile_pool(name="const", bufs=1))
            wst = ctx.enter_context(tc.tile_pool(name="wst", bufs=2))
            acts = ctx.enter_context(tc.tile_pool(name="acts", bufs=3))
            hts = ctx.enter_context(tc.tile_pool(name="hts", bufs=2))
            gpool = ctx.enter_context(tc.tile_pool(name="gpool", bufs=2))
            outs = ctx.enter_context(tc.tile_pool(name="outs", bufs=2))
            psum = ctx.enter_context(
                tc.tile_pool(name="psum", bufs=3, space="PSUM"))
            psg = ctx.enter_context(
                tc.tile_pool(name="psg", bufs=2, space="PSUM"))

            state = {}

            def load_xp(e):
                XP = acts.tile([128, 8, cap], bf, tag="xp", name=f"xp{e}")
                nc.sync.dma_start(
                    out=XP, in_=xp_r[:, :, e * cap:(e + 1) * cap])
                state[("XP", e)] = XP

            def load_w(e, split=False):
                W1 = wst.tile([128, 8, FL], bf, tag="w1", name=f"w1_{e}")
                W3 = wst.tile([128, 8, FL], bf, tag="w3", name=f"w3_{e}")
                w1r = w1l[e].rearrange("(a p) f -> p a f", p=128)
                w3r = w3l[e].rearrange("(a p) f -> p a f", p=128)
                if split:  # f-subtile split so the first MMs can start early
                    for ft in range(4):
                        fsl = slice(ft * 128, (ft + 1) * 128)
                        nc.sync.dma_start(out=W1[:, :, fsl],
                                          in_=w1r[:, :, fsl])
                        nc.sync.dma_start(out=W3[:, :, fsl],
                                          in_=w3r[:, :, fsl])
                else:
                    nc.sync.dma_start(out=W1, in_=w1r)
                    nc.sync.dma_start(out=W3, in_=w3r)
                W2 = wst.tile([128, 4, D], bf, tag="w2", name=f"w2_{e}")
                nc.sync.dma_start(
                    out=W2, in_=w2l[e].rearrange("(a p) d -> p a d", p=128))
                state[("W", e)] = (W1, W3, W2)

            def load_xf(ch):
                o = ch * 512
                XF = acts.tile([128, 8, 512], bf, tag="xf", name=f"xf{ch}")
                nc.sync.dma_start(out=XF, in_=xf_r[:, :, o:o + 512])
                state[("XF", ch)] = XF

            def gate(e):
                """G[:, j] = (1-alpha) * softmax(logits[:, j])[e], bf16."""
                GW, SEL = state["GW"], state["SEL"]
                XP = state[("XP", e)]
                Ge = gpool.tile([128, cap], bf, tag="G", name=f"G{e}")
                DEN = gpool.tile([8, cap], f32, tag="den", name=f"dn{e}")
                NUM = gpool.tile([8, cap], f32, tag="num", name=f"nm{e}")
                for ci, (o, n) in enumerate(chunks):
                    lg = psg.tile([8, 512], f32, tag="lg", name=f"lg{e}_{ci}")
                    for dt in range(8):
                        nc.tensor.matmul(
                            lg[:, :n], GW[:, dt, :], XP[:, dt, o:o + n],
                            start=(dt == 0), stop=(dt == 7))
                    EXPt = gpool.tile([8, 512], f32, tag="exp",
                                      name=f"ex{e}_{ci}")
                    nc.scalar.activation(EXPt[:, :n], lg[:, :n], Act.Exp)
                    TMP = gpool.tile([8, 512], f32, tag="tmp",
                                     name=f"tm{e}_{ci}")
                    nc.vector.tensor_scalar_mul(TMP[:, :n], EXPt[:, :n],
                                                SEL[:, e:e + 1])
                    nc.gpsimd.partition_all_reduce(
                        DEN[:, o:o + n], EXPt[:, :n], channels=8,
                        reduce_op=bass_isa.ReduceOp.add)
                    nc.gpsimd.partition_all_reduce(
                        NUM[:, o:o + n], TMP[:, :n], channels=8,
                        reduce_op=bass_isa.ReduceOp.add)
                rden = gpool.tile([1, cap], f32, tag="rden", name=f"rd{e}")
                nc.vector.reciprocal(rden, DEN[0:1, :])
                grow = gpool.tile([1, cap], bf, tag="grow", name=f"gr{e}")
                nc.vector.scalar_tensor_tensor(
                    grow, NUM[0:1, :], 1.0 - ALPHA,
                    rden, Alu.mult, Alu.mult)
                nc.gpsimd.partition_broadcast(Ge, grow)
                state[("G", e)] = Ge

            def ffn13(e):
                XP = state[("XP", e)]
                W1, W3, _ = state[("W", e)]
                Ge = state[("G", e)]
                HT = hts.tile([128, 4, cap], bf, tag="ht", name=f"ht{e}")
                for ft in range(4):
                    p1 = psum.tile([128, 1024], f32, tag="ps",
                                   name=f"p1_{e}_{ft}")
                    p3 = psum.tile([128, 1024], f32, tag="ps",
                                   name=f"p3_{e}_{ft}")
                    for dt in range(8):
                        st, sp = dt == 0, dt == 7
                        lw1 = W1[:, dt, ft * 128:(ft + 1) * 128]
                        for (o, n) in chunks:
                            nc.tensor.matmul(p1[:, o:o + n], lw1,
                                             XP[:, dt, o:o + n],
                                             start=st, stop=sp)
                        lw3 = W3[:, dt, ft * 128:(ft + 1) * 128]
                        for (o, n) in chunks:
                            nc.tensor.matmul(p3[:, o:o + n], lw3,
                                             XP[:, dt, o:o + n],
                                             start=st, stop=sp)
                    for (o, n) in chunks:
                        sa = gpool.tile([128, 512], f32, tag="silu",
                                        name=f"sa{e}_{ft}_{o}")
                        nc.scalar.activation(sa[:, :n], p1[:, o:o + n],
                                             Act.Silu)
                        nc.vector.tensor_mul(HT[:, ft, o:o + n], sa[:, :n],
                                             p3[:, o:o + n])
                        nc.vector.tensor_mul(HT[:, ft, o:o + n],
                                             HT[:, ft, o:o + n],
                                             Ge[:, o:o + n])
                state[("HT", e)] = HT

            def mm2(e):
                W2 = state[("W", e)][2]
                HT = state[("HT", e)]
                yo = outs.tile([128, 8, cap], bf, tag="yo", name=f"yo{e}")
                for dt in range(8):
                    py = psum.tile([128, 1024], f32, tag="ps",
                                   name=f"py{e}_{dt}")
                    for ft in range(4):
                        st, sp = ft == 0, ft == 3
                        lw2 = W2[:, ft, dt * 128:(dt + 1) * 128]
                        for (o, n) in chunks:
                            nc.tensor.matmul(py[:, o:o + n], lw2,
                                             HT[:, ft, o:o + n],
                                             start=st, stop=sp)
                    if yo_act:
                        nc.scalar.activation(yo[:, dt, :], py[:, 0:cap],
                                             Act.Copy)
                    else:
                        nc.vector.tensor_copy(out=yo[:, dt, :],
                                              in_=py[:, 0:cap])
                nc.sync.dma_start(
                    out=yr_r[:, :, e * cap:(e + 1) * cap], in_=yo)

            def shared_chunk(ch):
                W1S, W3S, W2S = state["W1S"], state["W3S"], state["W2S"]
                o = ch * 512
                XF = state[("XF", ch)]
                HS = hts.tile([128, 4, 512], bf, tag="hs", name=f"hs{ch}")
                for ft in range(4):
                    p1 = psum.tile([128, 1024], f32, tag="ps",
                                   name=f"sp1_{ch}_{ft}")
                    p3 = psum.tile([128, 1024], f32, tag="ps",
                                   name=f"sp3_{ch}_{ft}")
                    for dt in range(8):
                        st, sp = dt == 0, dt == 7
                        nc.tensor.matmul(p1[:, 0:512],
                                         W1S[:, dt, ft * 128:(ft + 1) * 128],
                                         XF[:, dt, :], start=st, stop=sp)
                        nc.tensor.matmul(p3[:, 0:512],
                                         W3S[:, dt, ft * 128:(ft + 1) * 128],
                                         XF[:, dt, :], start=st, stop=sp)
                    sa = gpool.tile([128, 512], f32, tag="silu",
                                    name=f"ssa{ch}_{ft}")
                    nc.scalar.activation(sa, p1[:, 0:512], Act.Silu)
                    nc.vector.tensor_mul(HS[:, ft, :], sa, p3[:, 0:512])
                so = outs.tile([128, 8, 512], bf, tag="so", name=f"so{ch}")
                for dt in range(8):
                    py = psum.tile([128, 1024], f32, tag="ps",
                                   name=f"spy{ch}_{dt}")
                    for ft in range(4):
                        nc.tensor.matmul(py[:, 0:512],
                                         W2S[:, ft, dt * 128:(dt + 1) * 128],
                                         HS[:, ft, :],
                                         start=(ft == 0), stop=(ft == 3))
                    nc.scalar.activation(so[:, dt, :], py[:, 0:512], Act.Copy)
                nc.sync.dma_start(out=ys_r[:, :, o:o + 512], in_=so)

            # ---- prologue: DMAs in consumption order -------------------
            load_xf(0)
            W1S = const.tile([128, 8, FS], bf)
            nc.sync.dma_start(out=W1S, in_=w1s_r)
            W3S = const.tile([128, 8, FS], bf)
            nc.sync.dma_start(out=W3S, in_=w3s_r)
            W2S = const.tile([128, 4, D], bf)
            nc.sync.dma_start(out=W2S, in_=w2s_r)
            GW = const.tile([128, 8, E], bf)
            nc.sync.dma_start(out=GW, in_=gw_r)
            SEL = const.tile([E, E], f32)
            nc.sync.dma_start(out=SEL, in_=sel)
            state.update(W1S=W1S, W3S=W3S, W2S=W2S, GW=GW, SEL=SEL)
            load_xp(0)
            load_w(0, split=True)

            shared_chunk(0)      # fills the PE while expert-0 inputs stream
            gate(0)
            load_xp(1)
            load_w(1)
            load_xf(1)
            for e in range(E):
                if e + 2 < E:
                    load_xp(e + 2)
                if e + 1 < E:
                    load_w(e + 1)
                if e in (0, 2):
                    load_xf(e // 2 + 2)
                if gate_first and e + 1 < E:
                    gate(e + 1)
                ffn13(e)
                if (not gate_first) and e + 1 < E:
                    gate(e + 1)
                mm2(e)
                if e in (1, 3, 5):
                    shared_chunk((e + 1) // 2)

    nc.compile()
    return nc


def _get_program():
    global _NC
    if _NC is None:
        _NC = _build_program(KCFG)
    return _NC


def kernel(hidden_states, gate_W, w1_e, w3_e, w2_e, w1_s, w3_s, w2_s):
    global LAST_RESULT
    x = np.ascontiguousarray(np.asarray(hidden_states, np.float32).reshape(T, D))

    # ---- host routing (sharding decision) ---------------------------
    gate_W = np.asarray(gate_W, np.float32)
    logits = x @ gate_W.T                       # [T, E]
    m = logits.max(axis=1, keepdims=True)
    p = np.exp(logits - m)
    probs = p / p.sum(axis=1, keepdims=True)
    order = np.argsort(-probs, axis=1, kind="stable")[:, :K]   # [T, K]

    idx = []            # token indices routed to each expert
    for e in range(E):
        te = np.where((order == e).any(axis=1))[0]
        if len(te) > CAP:   # graceful over-capacity: keep highest-prob tokens
            keep = np.argsort(-probs[te, e], kind="stable")[:CAP]
            te = np.sort(te[keep])
        idx.append(te)

    # ---- build device inputs ----------------------------------------
    xT = np.ascontiguousarray(x.T)              # [D, T] fp32
    xf_bf = xT.astype(BF16)                     # [D, T]
    xp_bf = np.zeros((D, NPAIR), dtype=BF16)
    for e in range(E):
        te = idx[e]
        xp_bf[:, e * CAP: e * CAP + len(te)] = xf_bf[:, te]

    gw_bf = np.ascontiguousarray(gate_W.T).astype(BF16)      # [D, E]
    w1_e = np.asarray(w1_e, np.float32)
    w3_e = np.asarray(w3_e, np.float32)
    w2_e = np.asarray(w2_e, np.float32)
    w1_s = np.asarray(w1_s, np.float32)
    w3_s = np.asarray(w3_s, np.float32)
    # fold alpha/NS (an exact power of two) into the shared down-proj
    w2_s = np.asarray(w2_s, np.float32) * (ALPHA / NS)

    nc = _get_program()
    in_maps = []
    for c in range(N_CORES):
        fsl = slice(c * FL, (c + 1) * FL)
        in_maps.append({
            "xp": xp_bf,
            "xf": xf_bf,
            "gw": gw_bf,
            "sel": np.eye(E, dtype=np.float32),
            "w1l": np.ascontiguousarray(w1_e[:, :, fsl]).astype(BF16),
            "w3l": np.ascontiguousarray(w3_e[:, :, fsl]).astype(BF16),
            "w2l": np.ascontiguousarray(w2_e[:, fsl, :]).astype(BF16),
            "w1s": w1_s[c].astype(BF16),
            "w3s": w3_s[c].astype(BF16),
            "w2s": w2_s[c].astype(BF16),
        })

    res = run_bass_kernel_spmd(nc, in_maps, list(range(N_CORES)))
    LAST_RESULT = res

    # ---- host combine (unshard) -------------------------------------
    outT = np.zeros((D, T), np.float32)
    yr_sum = np.zeros((D, NPAIR), np.float32)
    for c in range(N_CORES):
        yr_sum += res.results[c]["yr"].astype(np.float32)
        outT += res.results[c]["ys"].astype(np.float32)
    for e in range(E):
        te = idx[e]
        outT[:, te] += yr_sum[:, e * CAP: e * CAP + len(te)]

    return np.ascontiguousarray(outT.T).reshape(B, S, D).astype(np.float32)



# revision 4
# speedup vs baseline: 1.1402x; 1.1402x over previous
"""DeepSeekMoE Trainium2 kernel (8 NeuronCores, SPMD, expert-parallel).

Strategy:
  - Host computes top-2 routing AND the gate values (it needs softmax probs
    for the routing decision anyway).  Gate x (1-alpha) is applied on the
    host during the scatter-add of per-pair outputs, so the device computes
    UNGATED expert FFNs only -- no gate matmul/softmax machinery on device.
  - Expert parallel: core c holds routed expert c's full weights [D,F] and
    processes only the tokens routed to expert c (padded to a uniform
    capacity CAP so all cores run the same SPMD program).  Its yr output is
    final for those pairs (no cross-core reduction of routed outputs).
  - Shared experts are sharded 1:1: core c runs shared expert c over all
    T tokens; the host sums the 8 contributions (alpha/NS folded into w2_s).
    Shared chunks are interleaved between routed f-tiles to cover DMA ramps
    and the mm2 output tail.
  - All matmuls bf16 into fp32 PSUM.  Weights/activations are pre-packed on
    the host into [128, a, f] tile layouts so every DMA line is contiguous.
"""

import numpy as np
import ml_dtypes

import concourse.bacc as bacc
import concourse.tile as tile
import concourse.mybir as mybir
from concourse.bass_utils import run_bass_kernel_spmd

BF16 = ml_dtypes.bfloat16

B, S, D, F, E, NS, K = 2, 1024, 1024, 4096, 8, 8, 2
T = B * S
FS = F // NS            # shared expert hidden = 512
ALPHA = 0.5
N_CORES = 8
NFT = F // 128          # 32 f-tiles of the routed expert
NDT = D // 128          # 8 d-tiles

_NC = {}            # cap -> compiled program
LAST_RESULT = None  # BassKernelResults of the most recent run (for profiling)


def _build_program(cap):
    bf = mybir.dt.bfloat16
    f32 = mybir.dt.float32
    Act = mybir.ActivationFunctionType
    c0 = min(512, cap)
    chunks = [(0, c0)] + ([(512, cap - 512)] if cap > 512 else [])

    nc = bacc.Bacc("TRN2", target_bir_lowering=False, debug=False,
                   num_devices=N_CORES)

    # all inputs pre-packed host-side into [128, a, f] tile layout
    xp = nc.dram_tensor("xp", [128, NDT, cap], bf, kind="ExternalInput").ap()
    xf = nc.dram_tensor("xf", [128, NDT, T], bf, kind="ExternalInput").ap()
    w1 = nc.dram_tensor("w1", [NFT, 128, NDT, 128], bf,
                        kind="ExternalInput").ap()
    w3 = nc.dram_tensor("w3", [NFT, 128, NDT, 128], bf,
                        kind="ExternalInput").ap()
    w2 = nc.dram_tensor("w2", [NDT, 128, NFT, 128], bf,
                        kind="ExternalInput").ap()
    w1s = nc.dram_tensor("w1s", [128, NDT, FS], bf, kind="ExternalInput").ap()
    w3s = nc.dram_tensor("w3s", [128, NDT, FS], bf, kind="ExternalInput").ap()
    w2s = nc.dram_tensor("w2s", [128, FS // 128, D], bf,
                         kind="ExternalInput").ap()
    yr = nc.dram_tensor("yr", [128, NDT, cap], bf, kind="ExternalOutput").ap()
    ys = nc.dram_tensor("ys", [128, NDT, T], bf, kind="ExternalOutput").ap()

    with tile.TileContext(nc) as tc:
        with tc.tile_pool(name="const", bufs=1) as const, \
             tc.tile_pool(name="wst", bufs=6) as wst, \
             tc.tile_pool(name="w2st", bufs=4) as w2st, \
             tc.tile_pool(name="acts", bufs=1) as acts, \
             tc.tile_pool(name="xfp", bufs=2) as xfp, \
             tc.tile_pool(name="hts", bufs=2) as hts, \
             tc.tile_pool(name="spool", bufs=3) as spool, \
             tc.tile_pool(name="outs", bufs=2) as outs, \
             tc.tile_pool(name="psum", bufs=4, space="PSUM") as psum:

            state = {}

            def load_w13(ft):
                W1 = wst.tile([128, NDT, 128], bf, tag="w1", name=f"w1_{ft}")
                nc.sync.dma_start(out=W1, in_=w1[ft])
                W3 = wst.tile([128, NDT, 128], bf, tag="w3", name=f"w3_{ft}")
                nc.sync.dma_start(out=W3, in_=w3[ft])
                state[("W13", ft)] = (W1, W3)

            def load_w2(dt):
                W2 = w2st.tile([128, NFT, 128], bf, tag="w2", name=f"w2_{dt}")
                nc.sync.dma_start(out=W2, in_=w2[dt])
                state[("W2", dt)] = W2

            def load_xf(ch):
                XF = xfp.tile([128, NDT, 512], bf, tag="xf", name=f"xf{ch}")
                nc.sync.dma_start(out=XF, in_=xf[:, :, ch * 512:(ch + 1) * 512])
                state[("XF", ch)] = XF

            def ffn13(ft):
                """h[ft] = silu(w1[ft]^T xp) * (w3[ft]^T xp), bf16."""
                XP, HT = state["XP"], state["HT"]
                W1, W3 = state.pop(("W13", ft))
                p1 = psum.tile([128, 1024], f32, tag="ps", name=f"p1_{ft}")
                p3 = psum.tile([128, 1024], f32, tag="ps", name=f"p3_{ft}")
                for dt in range(NDT):
                    st, sp = dt == 0, dt == NDT - 1
                    for (o, n) in chunks:
                        nc.tensor.matmul(p1[:, o:o + n], W1[:, dt, :],
                                         XP[:, dt, o:o + n], start=st, stop=sp)
                    for (o, n) in chunks:
                        nc.tensor.matmul(p3[:, o:o + n], W3[:, dt, :],
                                         XP[:, dt, o:o + n], start=st, stop=sp)
                for (o, n) in chunks:
                    sa = spool.tile([128, 512], f32, tag="silu",
                                    name=f"sa{ft}_{o}")
                    nc.scalar.activation(sa[:, :n], p1[:, o:o + n], Act.Silu)
                    nc.vector.tensor_mul(HT[:, ft, o:o + n], sa[:, :n],
                                         p3[:, o:o + n])

            def mm2(dt):
                HT = state["HT"]
                W2 = state.pop(("W2", dt))
                yo = outs.tile([128, cap], bf, tag="yo", name=f"yo{dt}")
                py = psum.tile([128, 1024], f32, tag="ps", name=f"py{dt}")
                for ft in range(NFT):
                    st, sp = ft == 0, ft == NFT - 1
                    for (o, n) in chunks:
                        nc.tensor.matmul(py[:, o:o + n], W2[:, ft, :],
                                         HT[:, ft, o:o + n], start=st, stop=sp)
                nc.scalar.activation(yo, py[:, 0:cap], Act.Copy)
                nc.sync.dma_start(out=yr[:, dt, :], in_=yo)

            def shared_chunk(ch):
                W1S, W3S, W2S = state["W1S"], state["W3S"], state["W2S"]
                XF = state.pop(("XF", ch))
                HS = hts.tile([128, FS // 128, 512], bf, tag="hs",
                              name=f"hs{ch}")
                for ft in range(FS // 128):
                    p1 = psum.tile([128, 1024], f32, tag="ps",
                                   name=f"sp1_{ch}_{ft}")
                    p3 = psum.tile([128, 1024], f32, tag="ps",
                                   name=f"sp3_{ch}_{ft}")
                    for dt in range(NDT):
                        st, sp = dt == 0, dt == NDT - 1
                        nc.tensor.matmul(p1[:, 0:512],
                                         W1S[:, dt, ft * 128:(ft + 1) * 128],
                                         XF[:, dt, :], start=st, stop=sp)
                        nc.tensor.matmul(p3[:, 0:512],
                                         W3S[:, dt, ft * 128:(ft + 1) * 128],
                                         XF[:, dt, :], start=st, stop=sp)
                    sa = spool.tile([128, 512], f32, tag="silu",
                                    name=f"ssa{ch}_{ft}")
                    nc.scalar.activation(sa, p1[:, 0:512], Act.Silu)
                    nc.vector.tensor_mul(HS[:, ft, :], sa, p3[:, 0:512])
                so = outs.tile([128, NDT, 512], bf, tag="so", name=f"so{ch}")
                for dt in range(NDT):
                    py = psum.tile([128, 1024], f32, tag="ps",
                                   name=f"spy{ch}_{dt}")
                    for ft in range(FS // 128):
                        nc.tensor.matmul(py[:, 0:512],
                                         W2S[:, ft, dt * 128:(dt + 1) * 128],
                                         HS[:, ft, :],
                                         start=(ft == 0), stop=(ft == 3))
                    nc.scalar.activation(so[:, dt, :], py[:, 0:512], Act.Copy)
                nc.sync.dma_start(out=ys[:, :, ch * 512:(ch + 1) * 512],
                                  in_=so)

            # ---- prologue: DMAs in consumption order -------------------
            XP = acts.tile([128, NDT, cap], bf)
            nc.sync.dma_start(out=XP, in_=xp)
            HT = hts.tile([128, NFT, cap], bf, tag="ht")
            state.update(XP=XP, HT=HT)
            for ft in range(3):
                load_w13(ft)
            load_xf(0)
            W1S = const.tile([128, NDT, FS], bf)
            nc.sync.dma_start(out=W1S, in_=w1s)
            W3S = const.tile([128, NDT, FS], bf)
            nc.sync.dma_start(out=W3S, in_=w3s)
            W2S = const.tile([128, FS // 128, D], bf)
            nc.sync.dma_start(out=W2S, in_=w2s)
            state.update(W1S=W1S, W3S=W3S, W2S=W2S)

            # ---- main schedule ----------------------------------------
            # routed f-tiles with shared chunks interleaved; mm2 last,
            # followed by the final shared chunk to cover the yr tail.
            for ft in range(NFT):
                if ft + 3 < NFT:
                    load_w13(ft + 3)
                elif ft + 3 < NFT + NDT:      # prefetch w2 dt-slabs
                    load_w2(ft + 3 - NFT)
                ffn13(ft)
                if ft in (9, 19):
                    load_xf(ft // 10 + 1)
                if ft in (10, 20, 30):
                    shared_chunk(ft // 10 - 1)
            for dt in range(NDT):
                if dt + 3 < NDT:
                    load_w2(dt + 3)
                if dt == 0:
                    load_xf(3)
                mm2(dt)
            shared_chunk(3)

    nc.compile()
    return nc


def _get_program(cap):
    if cap not in _NC:
        _NC[cap] = _build_program(cap)
    return _NC[cap]


def _pack_dff_tiles(w):
    """[D, Fx] -> [Fx/128, 128, D/128, 128] tiles: t[ft, p, a, f]."""
    d, fx = w.shape
    return np.ascontiguousarray(
        w.reshape(d // 128, 128, fx // 128, 128).transpose(2, 1, 0, 3))


def _pack_part(w):
    """[D, N] -> [128, D/128, N]: t[p, a, n] = w[a*128+p, n]."""
    d, n = w.shape
    return np.ascontiguousarray(w.reshape(d // 128, 128, n).transpose(1, 0, 2))


def _unpack_part(t):
    """[128, A, N] -> [A*128, N]."""
    p, a, n = t.shape
    return t.transpose(1, 0, 2).reshape(a * p, n)


def kernel(hidden_states, gate_W, w1_e, w3_e, w2_e, w1_s, w3_s, w2_s):
    global LAST_RESULT
    x = np.ascontiguousarray(
        np.asarray(hidden_states, np.float32).reshape(T, D))

    # ---- host routing + gate values ---------------------------------
    gate_W = np.asarray(gate_W, np.float32)
    logits = x @ gate_W.T                       # [T, E]
    m = logits.max(axis=1, keepdims=True)
    p = np.exp(logits - m)
    probs = p / p.sum(axis=1, keepdims=True)
    order = np.argsort(-probs, axis=1, kind="stable")[:, :K]   # [T, K]

    idx = []            # token indices routed to each expert
    for e in range(E):
        te = np.where((order == e).any(axis=1))[0]
        idx.append(te)
    cap = max(544, -(-max(len(te) for te in idx) // 32) * 32)

    # ---- build device inputs ----------------------------------------
    xT = np.ascontiguousarray(x.T)              # [D, T] fp32
    xf_bf = _pack_part(xT.astype(BF16))         # [128, 8, T]

    w1_e = np.asarray(w1_e, np.float32)
    w3_e = np.asarray(w3_e, np.float32)
    w2_e = np.asarray(w2_e, np.float32)
    w1_s = np.asarray(w1_s, np.float32)
    w3_s = np.asarray(w3_s, np.float32)
    # fold alpha/NS (an exact power of two) into the shared down-proj
    w2_s = np.asarray(w2_s, np.float32) * (ALPHA / NS)

    nc = _get_program(cap)
    in_maps = []
    for c in range(N_CORES):
        te = idx[c]
        xp = np.zeros((D, cap), dtype=BF16)
        xp[:, :len(te)] = xT[:, te].astype(BF16)
        in_maps.append({
            "xp": _pack_part(xp),
            "xf": xf_bf,
            "w1": _pack_dff_tiles(w1_e[c].astype(BF16)),
            "w3": _pack_dff_tiles(w3_e[c].astype(BF16)),
            "w2": _pack_dff_tiles(w2_e[c].astype(BF16)),
            "w1s": _pack_part(w1_s[c].astype(BF16)),
            "w3s": _pack_part(w3_s[c].astype(BF16)),
            "w2s": _pack_part(w2_s[c].astype(BF16)),
        })

    res = run_bass_kernel_spmd(nc, in_maps, list(range(N_CORES)))
    LAST_RESULT = res

    # ---- host combine: gate-weighted scatter of yr + sum of ys ------
    outT = np.zeros((D, T), np.float32)
    for c in range(N_CORES):
        outT += _unpack_part(res.results[c]["ys"].astype(np.float32))
        te = idx[c]
        yrc = _unpack_part(res.results[c]["yr"].astype(np.float32))
        gate = (1.0 - ALPHA) * probs[te, c]
        outT[:, te] += yrc[:, :len(te)] * gate[None, :]

    return np.ascontiguousarray(outT.T).reshape(B, S, D).astype(np.float32)
